# revision 7
# baseline (speedup 1.0000x reference)
"""Trainium2 Bass kernel for ContinuousIntegratedKoopmanOperator.

reference: odeint(dz/dt = z @ W) sampled at t = DT*[1..T], y0 = x at t[0].
Closed form (time-invariant linear ODE): out[:, j, :] = x @ expm(DT*j*W).

Strategy (v4, delta-fp8 rewrite of the 64.5us fp16 baseline):
  * device computes per-step DELTAS d_j = x @ (M^j - M^{j-1}), j=1..63,
    pre-scaled per-column (power-of-2) to ~unit std; stores fp8e3 (e3m4)
    -> 8.4MB/core stores. Host decodes + cumsums from exact f32 x, so
    per-step quantization errors random-walk: ~1.5e-3 rel total.
  * pipeline is rate-matched: the V+S drains (PSUM f32 -> fp8e3 at
    1 elem/lane/cycle, ~1.96 col/ns combined) are the hard consumer
    ceiling; the PE producer must run just BELOW that or either side
    blocks and pays 1.3-2.5us engine wake latency per block (the v2
    all-DoubleRow producer at 2.32 col/ns convoyed to 74us this way).
    Mix per tile: units 0-3 fp8e4 DoubleRow (217ns, x8+residual in the
    two k-tiles) + units 4-15 fp16 (290ns) -> ~1.88 col/ns.
  * drains in 2048-col quads (4 psum banks incl 8-col pads), greedy
    V/S split; V/S warmup copies sized so the first gated drain's sem
    is already satisfied when checked (no blocked-idle wake).
  * trailing dummy matmuls keep the PE busy through the store tail:
    HAM drops to k=4 (DMA crawls) ~3.6us after the PE goes idle.
  * 16 half-tile stores (512KB) so the DMA tail drains wide.
"""
import numpy as np
import ml_dtypes

DT = 0.01
B, D, T = 8192, 128, 64
NCORES = 8
BSH = B // NCORES          # 1024 rows per core
NTILES = BSH // 128        # 8 batch tiles per core
NJ = T - 1                 # j=1..63 on device; j=0 is x itself (host)
CT = NJ * D                # 8064 real output cols per row
UW = 504                   # matmul unit width (fits a 512-col psum bank)
NU_T = CT // UW            # 16 units per tile
NDR = 4                    # units 0..NDR-1 of each tile are DoubleRow fp8
DRC = NDR * UW             # 2016 fp8 md cols; [DRC:CT) are fp16
NUNITS = NTILES * NU_T     # 128 units
SLOT = 512                 # psum bank width (f32 cols); unit u -> bank u%8
PADW = SLOT * NU_T         # 8192 staged cols per tile (incl 8-col pads)
QUADS = NUNITS // 4        # 32 drain quads (4 banks = 2048 cols each)
DUMW = 17                  # leading PE warmup matmuls
TRAILD = 8                 # trailing dummies: hold HAM k=8 through store tail
TSTD = 1.4                 # target per-column std of device outputs (e3m4)

# static drain-quad engine assignment (greedy by finish time)
_DUR = {"V": 2283.0, "S": 1837.0}
ENG_OF, IDX_OF = [], []
_fin = {"V": 0.0, "S": 0.0}
_cnt = {"V": 0, "S": 0}
for _q in range(QUADS):
    _e = min(("S", "V"), key=lambda e: _fin[e] + _DUR[e])
    _fin[_e] += _DUR[_e]
    _cnt[_e] += 1
    ENG_OF.append(_e)
    IDX_OF.append(_cnt[_e])

_CACHE = {}


def _host_tables(W: np.ndarray):
    """float64 delta table -> (md8 [128, 2*DRC] fp8e4 ktile-dup,
    md16 [128, CT-DRC] f16, s2 f32 [NJ, D] decode scales)."""
    A = DT * W.astype(np.float64)
    M1 = np.eye(D)
    term = np.eye(D)
    for n in range(1, 30):
        term = term @ A / n
        M1 += term
    E = M1 - np.eye(D)
    Dp = np.empty((D, CT), dtype=np.float64)  # scaled deltas, j-major cols
    s2 = np.empty((NJ, D), dtype=np.float32)
    P = np.eye(D)                             # M^{j-1}
    for j in range(1, T):
        Dj = P @ E                            # M^{j-1} (M - I)
        P = P @ M1
        cn = np.linalg.norm(Dj, axis=0) / TSTD
        sc = np.exp2(np.round(np.log2(cn)))
        s2[j - 1] = sc.astype(np.float32)
        Dp[:, (j - 1) * D:j * D] = Dj / sc[None, :]
    m8 = Dp[:, :DRC].astype(ml_dtypes.float8_e4m3)
    md8 = np.empty((D, 2 * DRC), dtype=ml_dtypes.float8_e4m3)
    md8[:, :DRC] = m8
    md8[:, DRC:] = m8
    md16 = Dp[:, DRC:].astype(np.float16)
    return md8, md16, s2


def _build_nc():
    import concourse.bass as bass
    import concourse.mybir as mybir

    f32 = mybir.dt.float32
    f16 = mybir.dt.float16
    f8e4 = mybir.dt.float8e4
    f8e3 = mybir.dt.float8e3
    DR = mybir.MatmulPerfMode.DoubleRow

    nc = bass.Bass(trn_type="TRN2")
    xr_d = nc.dram_tensor("xr", (D, 2 * BSH), f8e4, kind="ExternalInput")
    xt_d = nc.dram_tensor("xt", (D, BSH), f16, kind="ExternalInput")
    md8_d = nc.dram_tensor("md8", (D, 2 * DRC), f8e4, kind="ExternalInput")
    md16_d = nc.dram_tensor("md16", (D, CT - DRC), f16, kind="ExternalInput")
    out_d = nc.dram_tensor("out8", (BSH, PADW), f8e3, kind="ExternalOutput")

    xr_s = nc.alloc_sbuf_tensor("xr_s", [D, 2, BSH], f8e4)
    xt_s = nc.alloc_sbuf_tensor("xt_s", [D, BSH], f16)
    md8_s = nc.alloc_sbuf_tensor("md8_s", [D, 2, DRC], f8e4)
    md16_s = nc.alloc_sbuf_tensor("md16_s", [D, CT - DRC], f16)
    stg = [nc.alloc_sbuf_tensor(f"stg{i}", [128, PADW], f8e3) for i in range(NTILES)]
    scr_v = nc.alloc_sbuf_tensor("scr_v", [128, 4224], f8e3)
    scr_s = nc.alloc_sbuf_tensor("scr_s", [128, 5824], f8e3)
    psum = nc.alloc_psum_tensor("acc", [128, 8 * SLOT], f32)

    s_ld = nc.alloc_semaphore("s_ld")
    s_mm = nc.alloc_semaphore("s_mm")
    s_dv = nc.alloc_semaphore("s_dv")
    s_da = nc.alloc_semaphore("s_da")
    s_out = nc.alloc_semaphore("s_out")
    s_boot = nc.alloc_semaphore("s_boot")
    all_sems = [s_ld, s_mm, s_dv, s_da, s_out, s_boot]
    nums = sorted(s.num for s in all_sems)
    assert nums == list(range(nums[0], nums[-1] + 1)), "sems not contiguous"
    sem_range = range(nums[0], nums[-1] + 1)
    nc.gpsimd.dma_reset(sem_range)

    def quad_wait(eng, q):
        eng.wait_ge(s_dv if ENG_OF[q] == "V" else s_da, IDX_OF[q])

    with nc.Block() as block:
        @block.sync
        def _(sync):
            sync.sem_clear(sem_range)
            sync.nop().then_inc(s_boot, 1)
            # loads ordered so tile-0 gates clear ASAP (s_ld += 16 each):
            # 1,2: xr8 tile0 (both k-tiles)   3,4: md8 (both k-tiles)
            # 5: xt16 tile0                   6: md16 chunk c1
            # 7,8: xr8 rest                   9: xt16 rest
            # 10,11: md16 chunks c2, c3
            sync.dma_start(out=xr_s[:, 0:1, 0:128], in_=xr_d[:, 0:128]).then_inc(s_ld, 16)
            sync.dma_start(out=xr_s[:, 1:2, 0:128], in_=xr_d[:, BSH:BSH + 128]).then_inc(s_ld, 16)
            for k in range(2):
                sync.dma_start(out=md8_s[:, k:k + 1, :],
                               in_=md8_d[:, k * DRC:(k + 1) * DRC]).then_inc(s_ld, 16)
            sync.dma_start(out=xt_s[:, 0:128], in_=xt_d[:, 0:128]).then_inc(s_ld, 16)
            sync.dma_start(out=md16_s[:, 0:DRC],
                           in_=md16_d[:, 0:DRC]).then_inc(s_ld, 16)
            sync.dma_start(out=xr_s[:, 0:1, 128:BSH], in_=xr_d[:, 128:BSH]).then_inc(s_ld, 16)
            sync.dma_start(out=xr_s[:, 1:2, 128:BSH], in_=xr_d[:, BSH + 128:2 * BSH]).then_inc(s_ld, 16)
            sync.dma_start(out=xt_s[:, 128:BSH], in_=xt_d[:, 128:BSH]).then_inc(s_ld, 16)
            for c in (1, 2):
                sync.dma_start(out=md16_s[:, c * DRC:(c + 1) * DRC],
                               in_=md16_d[:, c * DRC:(c + 1) * DRC]).then_inc(s_ld, 16)
            for st in range(2 * NTILES):
                q_hi = 2 * st + 1
                cv = sum(1 for q in range(q_hi + 1) if ENG_OF[q] == "V")
                ca = (q_hi + 1) - cv
                if cv:
                    sync.wait_ge(s_dv, cv)
                if ca:
                    sync.wait_ge(s_da, ca)
                t, h = st // 2, st % 2
                sync.dma_start(
                    out=out_d[t * 128:(t + 1) * 128, h * 4096:(h + 1) * 4096],
                    in_=stg[t][:, h * 4096:(h + 1) * 4096],
                ).then_inc(s_out, 16)
            sync.wait_ge(s_out, 16 * 2 * NTILES)

        @block.scalar
        def _(scalar):
            # warmup sized to end as the first gated drain's sem arrives
            scalar.copy(out=scr_s[:, 0:2912], in_=scr_s[:, 2912:5824])
            scalar.copy(out=scr_s[:, 0:2912], in_=scr_s[:, 2912:5824])
            scalar.wait_ge(s_boot, 1)
            for q in range(QUADS):
                if ENG_OF[q] != "S":
                    continue
                scalar.wait_ge(s_mm, 4 * q + 4)
                po = (4 * q % 8) * SLOT
                scalar.copy(out=stg[q // 4][:, (q % 4) * 2048:(q % 4 + 1) * 2048],
                            in_=psum[:, po:po + 2048]).then_inc(s_da, 1)

        @block.vector
        def _(vector):
            for _ in range(3):
                vector.tensor_copy(out=scr_v[:, 0:2112], in_=scr_v[:, 2112:4224])
            vector.wait_ge(s_boot, 1)
            for q in range(QUADS):
                if ENG_OF[q] != "V":
                    continue
                vector.wait_ge(s_mm, 4 * q + 4)
                po = (4 * q % 8) * SLOT
                vector.tensor_copy(out=stg[q // 4][:, (q % 4) * 2048:(q % 4 + 1) * 2048],
                                   in_=psum[:, po:po + 2048]).then_inc(s_dv, 1)

        @block.tensor
        def _(tensor):
            # leading dummies: warm the PE p-state + HAM through the NEFF
            # preamble/load window; 512-wide so every psum col (incl pads)
            # is initialized before any drain reads it.
            for k in range(DUMW):
                tensor.matmul(psum[:, (k % 8) * SLOT:(k % 8) * SLOT + SLOT],
                              xr_s[:, :, 0:128], md8_s[:, :, 0:SLOT],
                              start=True, stop=True, perf_mode=DR)
            tensor.wait_ge(s_boot, 1)
            for u in range(NUNITS):
                t, b = u // NU_T, u % NU_T
                if t == 0 and b in (0, 4, 8, 12):
                    tensor.wait_ge(s_ld, 16 * {0: 4, 4: 6, 8: 10, 12: 11}[b])
                if t == 1 and b == 0:
                    tensor.wait_ge(s_ld, 16 * 8)   # xr8 rest
                if t == 1 and b == 4:
                    tensor.wait_ge(s_ld, 16 * 9)   # xt16 rest
                if u >= 8 and u % 4 == 0:
                    quad_wait(tensor, (u - 8) // 4)
                po = (u % 8) * SLOT
                if b < NDR:
                    mm = tensor.matmul(psum[:, po:po + UW],
                                       xr_s[:, :, t * 128:(t + 1) * 128],
                                       md8_s[:, :, b * UW:(b + 1) * UW],
                                       start=True, stop=True, perf_mode=DR)
                else:
                    mm = tensor.matmul(psum[:, po:po + UW],
                                       xt_s[:, t * 128:(t + 1) * 128],
                                       md16_s[:, (b - NDR) * UW:(b - NDR + 1) * UW],
                                       start=True, stop=True)
                mm.then_inc(s_mm, 1)
            # trailing dummies: HAM drops DMA to k=4 ~3.6us after PE idles,
            # which crawls the store tail. Keep streaming garbage matmuls.
            for k in range(TRAILD):
                if k < 8 and k % 4 == 0:
                    quad_wait(tensor, 30 + k // 4)
                tensor.matmul(psum[:, (k % 8) * SLOT:(k % 8) * SLOT + SLOT],
                              xr_s[:, :, 0:128], md8_s[:, :, 0:SLOT],
                              start=True, stop=True, perf_mode=DR)

    return nc


def _prep_inputs(x: np.ndarray, md8, md16):
    """Per-core input maps: xr = [p][ktile*BSH + m] fp8e4 (x8^T, r8^T),
    xt = x^T f16."""
    maps = []
    for c in range(NCORES):
        xc = x[c * BSH:(c + 1) * BSH]                       # (BSH, D) f32
        x8 = xc.astype(ml_dtypes.float8_e4m3)
        r = xc - x8.astype(np.float32)
        r8 = r.astype(ml_dtypes.float8_e4m3)
        xr = np.empty((D, 2 * BSH), dtype=ml_dtypes.float8_e4m3)
        xr[:, :BSH] = x8.T
        xr[:, BSH:] = r8.T
        xt = np.ascontiguousarray(xc.T.astype(np.float16))
        maps.append({"xr": np.ascontiguousarray(xr), "xt": xt,
                     "md8": md8, "md16": md16})
    return maps


def run_on_device(x: np.ndarray, tables, trace: bool = False):
    from concourse.bass_utils import run_bass_kernel_spmd

    md8, md16, s2 = tables
    if "nc" not in _CACHE:
        _CACHE["nc"] = _build_nc()
    nc = _CACHE["nc"]

    in_maps = _prep_inputs(x, md8, md16)
    res = run_bass_kernel_spmd(nc, in_maps, core_ids=list(range(NCORES)), trace=trace)
    out = np.empty((B, T, D), dtype=np.float32)
    for c in range(NCORES):
        xc = x[c * BSH:(c + 1) * BSH].astype(np.float32)
        raw = res.results[c]["out8"].astype(np.float32)     # (BSH, PADW)
        d = raw.reshape(BSH, NU_T, SLOT)[:, :, :UW].reshape(BSH, NJ, D)
        d *= s2[None, :, :]
        out[c * BSH:(c + 1) * BSH, 0] = xc
        out[c * BSH:(c + 1) * BSH, 1:] = xc[:, None, :] + np.cumsum(d, axis=1)
    return out, res


def kernel(x, W, T):
    x = np.asarray(x, dtype=np.float32)
    W = np.asarray(W, dtype=np.float32)
    assert int(T) == 64 and x.shape == (B, D) and W.shape == (D, D)
    tables = _host_tables(W)
    out, _ = run_on_device(x, tables, trace=False)
    return out
